# revision 13
# baseline (speedup 1.0000x reference)
"""Trainium2 Bass kernel for nn_AutoSelectAttention (parametric Gaussian span scores).

Computes y[b,m,k] = -(((x[k] + mean[b,m]) / (softness[b,m] + EPS))**2) + intercept[b,m]
for x[k] = k - (L-1), k in [0, 2L-1).

Sharding: the fused batch*heads dim (32) is split 4-per-core across 8 NeuronCores;
each core's [4*1024, 2047] output band is independent (no collectives).

Per-core schedule (DMA-write-roofline bound, ~33.5 MB f32 out per core):
  - host precomputes per-token planes [mean, -1/(s+eps)^2, intercept] -> one
    small input DMA; its completion (~9.6us incl. fixed preamble) gates compute.
  - x grid [124, 2047] fp16 (exact for |int| <= 2048) built as one 512-col
    gpsimd iota + three DVE +const shifts so it's ready before the planes sem.
  - per block: ACT Square (z2 = (x+mean)^2, f32) then one DVE tensor_scalar
    (y = z2*ninv2 + intercept) into a grouped output tile.
  - blocks are 124 tokens tall, NOT 128: SBUF partitions map to SDMA engines
    as port = p//8, and engine 15 (partitions 120-127) is stochastically
    slowed by descriptor-ring/neighbor-NC port contention (observed 20-50%
    degradation on ~half the runs, up to +25us end-to-end).  With 124 rows,
    engine 15 serves only partitions 120-123 (half share), so it never
    becomes the critical engine even at half speed.  33 blocks cover tokens
    0..4091; the last 4 tokens per core are computed on host (0.1%).
  - output DRAM is group-contiguous (y1[3,124,W] singles, y2[15,124,2W]
    pairs): each group is one contiguous DRAM region with 8/16KB contiguous
    per-partition descriptors (16KB descriptors already run at ~426 GB/s,
    within 1% of the SBUF-fabric/write-side wall; pairs beat quads on
    scheduling because readiness is less lumpy).  Ramp single x3 then pairs
    keeps the DMA streaming gap-free from ~14us.
"""

import sys

import numpy as np

for _p in ("/opt/trn_rl_repo", "/root/.axon_site", "/opt/pypackages"):
    if _p not in sys.path:
        sys.path.append(_p)

L = 1024
W = 2 * L - 1  # 2047
BH = 32
M = 1024
EPS = 1e-5
NCORES = 8
BH_SH = BH // NCORES  # 4
ROWS = BH_SH * M  # 4096 tokens per core
PB = 124  # tokens per block (partitions 0..123; engine 15 gets a half share)
NBLK = ROWS // PB  # 33 full blocks on hardware
HW_ROWS = NBLK * PB  # 4092; tokens 4092..4095 are computed on host
GROUPS = [1, 1, 1] + [2] * 15
assert sum(GROUPS) == NBLK

_NC_CACHE = {}


def _build_nc():
    import concourse.bacc as bacc
    import concourse.tile as tile
    from concourse import mybir

    f32 = mybir.dt.float32
    f16 = mybir.dt.float16
    Sq = mybir.ActivationFunctionType.Square

    nc = bacc.Bacc("TRN2", target_bir_lowering=False, debug=False)
    # planes[p, 0, k] = mean, [p, 1, k] = -1/(softness+EPS)^2, [p, 2, k] =
    # intercept for token t = k*124 + p (host-precomputed).
    planes = nc.dram_tensor("planes", [PB, 3, NBLK], f32, kind="ExternalInput").ap()
    # One output tensor per group size; group i of size g occupies one fully
    # contiguous DRAM region laid out [partition, g*W] so every partition's
    # descriptor is g*8188 contiguous bytes and partitions are adjacent.
    n_by_g = {g: GROUPS.count(g) for g in set(GROUPS)}
    youts = {
        g: nc.dram_tensor(f"y{g}", [n, PB, g * W], f32, kind="ExternalOutput").ap()
        for g, n in sorted(n_by_g.items())
    }

    with tile.TileContext(nc) as tc:
        with (
            tc.tile_pool(name="const", bufs=1) as cpool,
            tc.tile_pool(name="work", bufs=4) as wpool,
            tc.tile_pool(name="o1", bufs=3) as o1pool,
            tc.tile_pool(name="o2", bufs=5) as o2pool,
        ):
            # Warmup ACTIVATE with no data dependencies: pulls the ~1.5us
            # Square table load to kernel start instead of serializing it
            # behind the planes DMA.
            warm = cpool.tile([PB, 1], f32)
            one = nc.const_aps.tensor(1.0, (PB, 1))
            nc.scalar.activation(warm[:], one, Sq, bias=0.0, scale=1.0)

            # x grid in fp16 (integers |x| <= 2047 are exact in fp16).  One
            # 512-wide iota on gpsimd, then DVE fills the rest with +const
            # shifts of the first chunk -- ready ~9.5us, just before the
            # planes DMA semaphore (~9.6us), where a full-width iota would
            # block z2 until ~11.9us.
            xb = cpool.tile([PB, 2 * L], f16)
            nc.gpsimd.iota(
                xb[:, 0:512],
                [[1, 512]],
                base=-(L - 1),
                channel_multiplier=0,
                allow_small_or_imprecise_dtypes=True,
            )
            for j in (1, 2, 3):
                nc.vector.tensor_scalar(
                    xb[:, j * 512 : (j + 1) * 512],
                    xb[:, 0:512],
                    float(j * 512),
                    None,
                    mybir.AluOpType.add,
                )

            spn = cpool.tile([PB, 3, NBLK], f32)
            nc.sync.dma_start(spn[:], planes[:, :, :])

            pools = {1: o1pool, 2: o2pool}
            gidx = {g: 0 for g in n_by_g}
            k = 0
            for g in GROUPS:
                ot = pools[g].tile([PB, g * W], f32)
                for j in range(g):
                    kk = k + j
                    # z2 = (x + mean)^2 on ACT (per-partition bias = mean)
                    z2 = wpool.tile([PB, W], f32)
                    nc.scalar.activation(
                        z2[:], xb[:, 0:W], Sq, bias=spn[:, 0, kk : kk + 1], scale=1.0
                    )
                    # y = z2 * ninv2 + intercept on DVE (per-partition scalars)
                    nc.vector.tensor_scalar(
                        ot[:, j * W : (j + 1) * W],
                        z2[:],
                        spn[:, 1, kk : kk + 1],
                        spn[:, 2, kk : kk + 1],
                        mybir.AluOpType.mult,
                        mybir.AluOpType.add,
                    )
                i = gidx[g]
                nc.sync.dma_start(youts[g][i : i + 1, :, :], ot[:])
                gidx[g] += 1
                k += g
    nc.compile()
    return nc


def _get_nc():
    if "nc" not in _NC_CACHE:
        _NC_CACHE["nc"] = _build_nc()
    return _NC_CACHE["nc"]


def _make_in_maps(span: np.ndarray) -> list[dict]:
    span = np.ascontiguousarray(span, dtype=np.float32)
    in_maps = []
    for c in range(NCORES):
        flat = span[c * BH_SH : (c + 1) * BH_SH].reshape(ROWS, 3)
        # [blk, p, comp] with token t = blk*124 + p (hardware tokens only)
        shard = flat[:HW_ROWS].reshape(NBLK, PB, 3)
        mean = shard[:, :, 0].T  # [p, blk]
        soft = shard[:, :, 1].T.astype(np.float64)
        cept = shard[:, :, 2].T
        ninv2 = (-1.0 / (soft + EPS) ** 2).astype(np.float32)
        planes = np.ascontiguousarray(
            np.stack([mean, ninv2, cept], axis=1), dtype=np.float32
        )  # [PB, 3, NBLK]
        in_maps.append({"planes": planes})
    return in_maps


def kernel(span: np.ndarray, _trace: bool = False, _tmpdir: str | None = None):
    from concourse.bass_utils import run_bass_kernel_spmd

    span = np.ascontiguousarray(span, dtype=np.float32)
    nc = _get_nc()
    in_maps = _make_in_maps(span)
    res = run_bass_kernel_spmd(
        nc,
        in_maps,
        core_ids=list(range(NCORES)),
        trace=_trace,
        tmpdir=_tmpdir,
    )
    # Host-side values for the 4-token remainder (tokens HW_ROWS..ROWS-1).
    x = (np.arange(W, dtype=np.float32) - np.float32(L - 1))[None, :]

    # Reassemble each core's [ROWS, W] band from the group-contiguous
    # tensors: group i of size g holds [PB, g, W] with token t = (k0+j)*124+p.
    shards = []
    for c, r in enumerate(res.results):
        band = np.empty((ROWS, W), np.float32)
        gidx = {g: 0 for g in set(GROUPS)}
        k = 0
        for g in GROUPS:
            i = gidx[g]
            arr = np.asarray(r[f"y{g}"]).reshape(-1, PB, g * W)[i]
            band[k * PB : (k + g) * PB, :] = (
                arr.reshape(PB, g, W).transpose(1, 0, 2).reshape(g * PB, W)
            )
            gidx[g] += 1
            k += g
        rem = span[c * BH_SH : (c + 1) * BH_SH].reshape(ROWS, 3)[HW_ROWS:]
        band[HW_ROWS:] = (
            -(((x + rem[:, 0:1]) / (rem[:, 1:2] + np.float32(EPS))) ** 2)
            + rem[:, 2:3]
        ).astype(np.float32)
        shards.append(band.reshape(BH_SH, M, W))
    out = np.concatenate(shards, axis=0).astype(np.float32)
    if _trace:
        kernel.last_results = res
    return out


# revision 14
# speedup vs baseline: 2.3710x; 2.3710x over previous
"""Trainium2 Bass kernel for nn_AutoSelectAttention (parametric Gaussian span scores).

Computes y[b,m,k] = -(((x[k] + mean[b,m]) / (softness[b,m] + EPS))**2) + intercept[b,m]
for x[k] = k - (L-1), k in [0, 2L-1).

Sharding: the fused batch*heads dim (32) is split 4-per-core across 8 NeuronCores;
each core's [4*1024, 2047] output band is independent (no collectives).

Per-core schedule (DMA-write-roofline bound, ~33.5 MB f32 out per core):
  - host precomputes per-token planes [mean, -1/(s+eps)^2, intercept] -> one
    small input DMA; its completion (~9.6us incl. fixed preamble) gates compute.
  - x grid fp16 (exact for |int| <= 2048) built as one 512-col gpsimd iota +
    three DVE +const shifts so it's ready before the planes semaphore.
  - per block: ACT Square (z2 = (x+mean)^2, f32) then one DVE tensor_scalar
    (y = z2*ninv2 + intercept) into a grouped output tile.
  - SDMA engine hazard: a DMA with P partitions is split evenly over the
    largest engine count n <= 16 with P % n == 0, 8 partitions per engine at
    P=128 (16 engines) / P=120 (15 engines, engine 15 unused).  Engine 15 is
    stochastically slowed 20-50% on ~half the runs (descriptor-ring /
    neighbor-NC port contention), which otherwise adds 10-35us.  Mixed block
    heights -- 17 blocks x 128 rows + 16 blocks x 120 rows = 4096 tokens
    exactly -- cut engine 15's share to 17/33, so it stops being the critical
    engine even at half speed, while the other 15 engines stay evenly loaded.
    Heights are interleaved (bb,aa,bb,aa,...) to spread engine 15's duty.
  - output DRAM is group-contiguous (y1a[3,128,W] singles, y2a[7,128,2W],
    y2b[8,120,2W] pairs): each group is one contiguous DRAM region with
    8/16KB contiguous per-partition descriptors (16KB descriptors run at
    ~426 GB/s, within 1% of the write-side wall; pairs beat quads because
    readiness is less lumpy).  Two singles first keep the DMA streaming
    gap-free from ~14us.
"""

import sys

import numpy as np

for _p in ("/opt/trn_rl_repo", "/root/.axon_site", "/opt/pypackages"):
    if _p not in sys.path:
        sys.path.append(_p)

L = 1024
W = 2 * L - 1  # 2047
BH = 32
M = 1024
EPS = 1e-5
NCORES = 8
BH_SH = BH // NCORES  # 4
ROWS = BH_SH * M  # 4096 tokens per core
H_A = 128  # tall blocks (all 16 SDMA engines)
H_B = 120  # short blocks (engines 0-14 only; engine 15 idle)

# Group sequence: two a-singles (early streaming), interleaved b/a pairs,
# trailing b-pair and a-single.  17 a-blocks + 16 b-blocks = 4096 tokens.
_SEQ = [("a", 1), ("a", 1)] + [("b", 2), ("a", 2)] * 7 + [("b", 2), ("a", 1)]


def _make_plan():
    plan = []
    base = 0
    slots = {"y1a": 0, "y2a": 0, "y2b": 0}
    for typ, g in _SEQ:
        h = H_A if typ == "a" else H_B
        key = "y1a" if g == 1 else ("y2a" if typ == "a" else "y2b")
        bases = []
        for _ in range(g):
            bases.append(base)
            base += h
        plan.append({"key": key, "slot": slots[key], "h": h, "g": g, "bases": bases})
        slots[key] += 1
    assert base == ROWS, base
    return plan, slots


PLAN, _SLOT_COUNTS = _make_plan()
NBLK = sum(p["g"] for p in PLAN)  # 33

_NC_CACHE = {}


def _build_nc():
    import concourse.bacc as bacc
    import concourse.tile as tile
    from concourse import mybir

    f32 = mybir.dt.float32
    f16 = mybir.dt.float16
    Sq = mybir.ActivationFunctionType.Square

    nc = bacc.Bacc("TRN2", target_bir_lowering=False, debug=False)
    # planes[p, 0, k] = mean, [p, 1, k] = -1/(softness+EPS)^2, [p, 2, k] =
    # intercept for block k (compute order), token = bases[k] + p, p < h_k.
    planes = nc.dram_tensor("planes", [H_A, 3, NBLK], f32, kind="ExternalInput").ap()
    youts = {
        "y1a": nc.dram_tensor(
            "y1a", [_SLOT_COUNTS["y1a"], H_A, W], f32, kind="ExternalOutput"
        ).ap(),
        "y2a": nc.dram_tensor(
            "y2a", [_SLOT_COUNTS["y2a"], H_A, 2 * W], f32, kind="ExternalOutput"
        ).ap(),
        "y2b": nc.dram_tensor(
            "y2b", [_SLOT_COUNTS["y2b"], H_B, 2 * W], f32, kind="ExternalOutput"
        ).ap(),
    }

    with tile.TileContext(nc) as tc:
        with (
            tc.tile_pool(name="const", bufs=1) as cpool,
            tc.tile_pool(name="work", bufs=3) as wpool,
            tc.tile_pool(name="o1", bufs=2) as o1pool,
            tc.tile_pool(name="o2a", bufs=3) as o2apool,
            tc.tile_pool(name="o2b", bufs=3) as o2bpool,
        ):
            # Warmup ACTIVATE with no data dependencies: pulls the ~1.5us
            # Square table load to kernel start instead of serializing it
            # behind the planes DMA.
            warm = cpool.tile([H_A, 1], f32)
            one = nc.const_aps.tensor(1.0, (H_A, 1))
            nc.scalar.activation(warm[:], one, Sq, bias=0.0, scale=1.0)

            # x grid in fp16 (integers |x| <= 2047 are exact in fp16).  One
            # 512-wide iota on gpsimd, then DVE fills the rest with +const
            # shifts of the first chunk -- ready ~9.5us, just before the
            # planes DMA semaphore, where a full-width iota would block z2
            # until ~11.9us.
            xb = cpool.tile([H_A, 2 * L], f16)
            nc.gpsimd.iota(
                xb[:, 0:512],
                [[1, 512]],
                base=-(L - 1),
                channel_multiplier=0,
                allow_small_or_imprecise_dtypes=True,
            )
            for j in (1, 2, 3):
                nc.vector.tensor_scalar(
                    xb[:, j * 512 : (j + 1) * 512],
                    xb[:, 0:512],
                    float(j * 512),
                    None,
                    mybir.AluOpType.add,
                )

            spn = cpool.tile([H_A, 3, NBLK], f32)
            nc.sync.dma_start(spn[:], planes[:, :, :])

            pools = {"y1a": o1pool, "y2a": o2apool, "y2b": o2bpool}
            k = 0
            for grp in PLAN:
                h, g = grp["h"], grp["g"]
                ot = pools[grp["key"]].tile([h, g * W], f32)
                for j in range(g):
                    kk = k + j
                    # z2 = (x + mean)^2 on ACT (per-partition bias = mean)
                    z2 = wpool.tile([H_A, W], f32)
                    nc.scalar.activation(
                        z2[0:h, :],
                        xb[0:h, 0:W],
                        Sq,
                        bias=spn[0:h, 0, kk : kk + 1],
                        scale=1.0,
                    )
                    # y = z2 * ninv2 + intercept on DVE (per-partition scalars)
                    nc.vector.tensor_scalar(
                        ot[:, j * W : (j + 1) * W],
                        z2[0:h, :],
                        spn[0:h, 1, kk : kk + 1],
                        spn[0:h, 2, kk : kk + 1],
                        mybir.AluOpType.mult,
                        mybir.AluOpType.add,
                    )
                i = grp["slot"]
                nc.sync.dma_start(youts[grp["key"]][i : i + 1, :, :], ot[:])
                k += g
    nc.compile()
    return nc


def _get_nc():
    if "nc" not in _NC_CACHE:
        _NC_CACHE["nc"] = _build_nc()
    return _NC_CACHE["nc"]


def _make_in_maps(span: np.ndarray) -> list[dict]:
    span = np.ascontiguousarray(span, dtype=np.float32)
    in_maps = []
    for c in range(NCORES):
        flat = span[c * BH_SH : (c + 1) * BH_SH].reshape(ROWS, 3)
        planes = np.zeros((H_A, 3, NBLK), np.float32)
        planes[:, 1, :] = -1.0  # harmless pad for rows >= h in short blocks
        k = 0
        for grp in PLAN:
            h = grp["h"]
            for j in range(grp["g"]):
                tok = flat[grp["bases"][j] : grp["bases"][j] + h]
                planes[:h, 0, k] = tok[:, 0]
                planes[:h, 1, k] = (
                    -1.0 / (tok[:, 1].astype(np.float64) + EPS) ** 2
                ).astype(np.float32)
                planes[:h, 2, k] = tok[:, 2]
                k += 1
        in_maps.append({"planes": np.ascontiguousarray(planes)})
    return in_maps


def kernel(span: np.ndarray, _trace: bool = False, _tmpdir: str | None = None):
    from concourse.bass_utils import run_bass_kernel_spmd

    span = np.ascontiguousarray(span, dtype=np.float32)
    nc = _get_nc()
    in_maps = _make_in_maps(span)
    res = run_bass_kernel_spmd(
        nc,
        in_maps,
        core_ids=list(range(NCORES)),
        trace=_trace,
        tmpdir=_tmpdir,
    )
    # Reassemble each core's [ROWS, W] band: group slot i holds [h, g, W]
    # with token t = bases[j] + p.
    shards = []
    for r in res.results:
        band = np.empty((ROWS, W), np.float32)
        for grp in PLAN:
            h, g, i = grp["h"], grp["g"], grp["slot"]
            arr = np.asarray(r[grp["key"]]).reshape(-1, h, g * W)[i]
            blocks = arr.reshape(h, g, W).transpose(1, 0, 2)  # [g, h, W]
            for j in range(g):
                b0 = grp["bases"][j]
                band[b0 : b0 + h, :] = blocks[j]
        shards.append(band.reshape(BH_SH, M, W))
    out = np.concatenate(shards, axis=0).astype(np.float32)
    if _trace:
        kernel.last_results = res
    return out


# revision 15
# speedup vs baseline: 3.2814x; 1.3840x over previous
"""Trainium2 Bass kernel for nn_AutoSelectAttention (parametric Gaussian span scores).

Computes y[b,m,k] = -(((x[k] + mean[b,m]) / (softness[b,m] + EPS))**2) + intercept[b,m]
for x[k] = k - (L-1), k in [0, 2L-1).

Sharding: the fused batch*heads dim (32) is split 4-per-core across 8 NeuronCores;
each core's [4*1024, 2047] output band is independent (no collectives).

Per-core schedule (DMA-write-roofline bound, ~33.5 MB f32 out per core):
  - host precomputes per-token planes [mean, -1/(s+eps)^2, intercept] -> one
    small input DMA; its completion (~9.6us incl. fixed preamble) gates compute.
  - x grid [128, 2047] fp16 (exact for |int| <= 2048) built as one 512-col
    gpsimd iota + three DVE +const shifts so it's ready before the planes
    semaphore (a full-width iota would block the first block until ~11.9us).
  - per 128-token block: ACT Square (z2 = (x+mean)^2, f32) then one DVE
    tensor_scalar (y = z2*ninv2 + intercept) into a grouped output tile.
  - output DRAM is split per group-size into group-contiguous tensors
    (y1[2,128,W], y2[3,128,2W], y4[6,128,4W]): each group is one fully
    contiguous 1/2/4MB DRAM region with 8/16/32KB contiguous per-partition
    descriptors.  Group ramp single,single,pair,pair,quad x6,pair starts the
    write stream at ~14us and runs it gap-free at ~430 GB/s (98% of the
    per-core write-side wall) in steady state.
  - all DMAs keep the full 128 partitions: the descriptor generator splits
    P partitions over the largest n <= 16 with P % n == 0, and non-16-engine
    configs (120/124-partition variants) measured 30-50% slower chip-wide
    under 8-core HBM contention.
"""

import sys

import numpy as np

for _p in ("/opt/trn_rl_repo", "/root/.axon_site", "/opt/pypackages"):
    if _p not in sys.path:
        sys.path.append(_p)

L = 1024
W = 2 * L - 1  # 2047
BH = 32
M = 1024
EPS = 1e-5
NCORES = 8
BH_SH = BH // NCORES  # 4
ROWS = BH_SH * M  # 4096 tokens per core
P = 128
NBLK = ROWS // P  # 32 blocks of 128 tokens

# Output DMA grouping ramp (must sum to NBLK): small groups early so the DMA
# starts streaming ASAP, quads in steady state for 32KB contiguous descriptors.
GROUPS = [1, 1, 2, 2, 4, 4, 4, 4, 4, 4, 2]
assert sum(GROUPS) == NBLK

_NC_CACHE = {}


def _build_nc():
    import concourse.bacc as bacc
    import concourse.tile as tile
    from concourse import mybir

    f32 = mybir.dt.float32
    f16 = mybir.dt.float16
    Sq = mybir.ActivationFunctionType.Square

    nc = bacc.Bacc("TRN2", target_bir_lowering=False, debug=False)
    # planes[p, 0, k] = mean, [p, 1, k] = -1/(softness+EPS)^2, [p, 2, k] =
    # intercept for token t = k*128 + p (host-precomputed).
    planes = nc.dram_tensor("planes", [P, 3, NBLK], f32, kind="ExternalInput").ap()
    # One output tensor per group size; group i of size g occupies one fully
    # contiguous g*1MB DRAM region laid out [partition, g*W] so every
    # partition's descriptor is g*8188 contiguous bytes and partitions are
    # adjacent (y*[i, p, j*W+w] = out[token (k0+j)*128+p, w]).
    n_by_g = {g: GROUPS.count(g) for g in set(GROUPS)}
    youts = {
        g: nc.dram_tensor(f"y{g}", [n, P, g * W], f32, kind="ExternalOutput").ap()
        for g, n in sorted(n_by_g.items())
    }

    with tile.TileContext(nc) as tc:
        with (
            tc.tile_pool(name="const", bufs=1) as cpool,
            tc.tile_pool(name="work", bufs=3) as wpool,
            tc.tile_pool(name="o1", bufs=2) as o1pool,
            tc.tile_pool(name="o2", bufs=3) as o2pool,
            tc.tile_pool(name="o4", bufs=2) as o4pool,
        ):
            # Warmup ACTIVATE with no data dependencies: pulls the ~1.5us
            # Square table load to kernel start instead of serializing it
            # behind the planes DMA.
            warm = cpool.tile([P, 1], f32)
            one = nc.const_aps.tensor(1.0, (P, 1))
            nc.scalar.activation(warm[:], one, Sq, bias=0.0, scale=1.0)

            # x grid in fp16 (integers |x| <= 2047 are exact in fp16).
            xb = cpool.tile([P, 2 * L], f16)
            nc.gpsimd.iota(
                xb[:, 0:512],
                [[1, 512]],
                base=-(L - 1),
                channel_multiplier=0,
                allow_small_or_imprecise_dtypes=True,
            )
            for j in (1, 2, 3):
                nc.vector.tensor_scalar(
                    xb[:, j * 512 : (j + 1) * 512],
                    xb[:, 0:512],
                    float(j * 512),
                    None,
                    mybir.AluOpType.add,
                )

            spn = cpool.tile([P, 3, NBLK], f32)
            nc.sync.dma_start(spn[:], planes[:, :, :])

            pools = {1: o1pool, 2: o2pool, 4: o4pool}
            gidx = {g: 0 for g in n_by_g}
            k = 0
            for g in GROUPS:
                ot = pools[g].tile([P, g * W], f32)
                for j in range(g):
                    kk = k + j
                    # z2 = (x + mean)^2 on ACT (per-partition bias = mean)
                    z2 = wpool.tile([P, W], f32)
                    nc.scalar.activation(
                        z2[:], xb[:, 0:W], Sq, bias=spn[:, 0, kk : kk + 1], scale=1.0
                    )
                    # y = z2 * ninv2 + intercept on DVE (per-partition scalars)
                    nc.vector.tensor_scalar(
                        ot[:, j * W : (j + 1) * W],
                        z2[:],
                        spn[:, 1, kk : kk + 1],
                        spn[:, 2, kk : kk + 1],
                        mybir.AluOpType.mult,
                        mybir.AluOpType.add,
                    )
                i = gidx[g]
                nc.sync.dma_start(youts[g][i : i + 1, :, :], ot[:])
                gidx[g] += 1
                k += g
    nc.compile()
    return nc


def _get_nc():
    if "nc" not in _NC_CACHE:
        _NC_CACHE["nc"] = _build_nc()
    return _NC_CACHE["nc"]


def _make_in_maps(span: np.ndarray) -> list[dict]:
    span = np.ascontiguousarray(span, dtype=np.float32)
    in_maps = []
    for c in range(NCORES):
        # [blk, p, comp] with token t = blk*128 + p
        shard = span[c * BH_SH : (c + 1) * BH_SH].reshape(NBLK, P, 3)
        mean = shard[:, :, 0].T  # [p, blk]
        soft = shard[:, :, 1].T.astype(np.float64)
        cept = shard[:, :, 2].T
        ninv2 = (-1.0 / (soft + EPS) ** 2).astype(np.float32)
        planes = np.ascontiguousarray(
            np.stack([mean, ninv2, cept], axis=1), dtype=np.float32
        )  # [128, 3, NBLK]
        in_maps.append({"planes": planes})
    return in_maps


def kernel(span: np.ndarray, _trace: bool = False, _tmpdir: str | None = None):
    from concourse.bass_utils import run_bass_kernel_spmd

    nc = _get_nc()
    in_maps = _make_in_maps(span)
    res = run_bass_kernel_spmd(
        nc,
        in_maps,
        core_ids=list(range(NCORES)),
        trace=_trace,
        tmpdir=_tmpdir,
    )
    # Reassemble each core's [ROWS, W] band from the group-contiguous
    # tensors: group i of size g holds [P, g, W] with token t = (k0+j)*128+p.
    shards = []
    for r in res.results:
        band = np.empty((ROWS, W), np.float32)
        gidx = {g: 0 for g in set(GROUPS)}
        k = 0
        for g in GROUPS:
            i = gidx[g]
            arr = np.asarray(r[f"y{g}"]).reshape(-1, P, g * W)[i]
            band[k * P : (k + g) * P, :] = (
                arr.reshape(P, g, W).transpose(1, 0, 2).reshape(g * P, W)
            )
            gidx[g] += 1
            k += g
        shards.append(band.reshape(BH_SH, M, W))
    out = np.concatenate(shards, axis=0).astype(np.float32)
    if _trace:
        kernel.last_results = res
    return out
